# revision 22
# baseline (speedup 1.0000x reference)
"""MiniMax-M2 decoder layer (attention + sigmoid-router top-2 MoE) on 8 TRN2 NeuronCores.

Sharding: head-parallel attention (2 q-heads + 1 kv-head per core), token-parallel
everything else (256 tokens per core).  x is replicated to every core as a
pre-transposed input (no AllGather); the ln1-rms row for all tokens is
accumulated from the same tiles the qkv loop streams.  Collectives: AllReduce
of qk-norm sumsq partials, AllToAll to reshard o^T from head- to
token-sharded.  Dense MoE over all 8 experts per token slice with bf16 weights
(combine weights zero unrouted experts).  The v projection gets the ln1-rms
row applied explicitly (q/k absorb it via the qk-norm scale rows).

Precision: the router's top-2 selection has tie gaps down to ~1.5e-5 in
sigmoid space, and the TRN2 PE rounds every fp32/fp32r product to ~12-15
mantissa bits, so no fp32-mode GEMM is accurate enough.  Everything feeding
x1 (qkv, scores, attn*v, o-proj, router) instead runs as hi/lo-compensated
FP16 matmuls (3-pass: wh*xh + wh*xl + wl*xh; fp16 products are exact in the
fp32 PSUM, giving ~2^-22 results at 3 cyc/row).  Weights are pre-scaled x64
on the host so fp16 residuals stay out of the subnormal range (unscaled on
PSUM evacuation); softmax numerators are scaled by 0.25 to stay under fp16
max (cancels in the ratio).  Row reductions (rmsnorm sumsq, softmax
denominators) use the ones-lhsT fp32r trick (values pass through the PE
exactly), and rsqrt rows get one Newton step.  RoPE tables are computed with
jax on CPU so they BIT-match the fp32 reference (a 1-ulp inv_freq difference
moves cos/sin by ~1e-4 at ~2000 rad and flips near-tie routing).  The MoE
itself stays bf16 (output tolerance 2e-2).

kernel(**inputs) takes full unsharded inputs, returns the full [1, S, D] output.
"""

import contextlib

import numpy as np
import ml_dtypes

import concourse.bass as bass
import concourse.mybir as mybir
import concourse.tile as tile
from concourse import bacc, bass_isa, bass_utils

F32 = mybir.dt.float32
F32R = mybir.dt.float32r
F16 = mybir.dt.float16
BF16 = mybir.dt.bfloat16
AF = mybir.ActivationFunctionType
OP = mybir.AluOpType
RG8 = [list(range(8))]

P = 128
D = 2048
H = 16
KVH = 4
DH = 128
RD = 64
E = 8
I = 1024
S = 2048
NCORE = 8
TPC = S // NCORE          # 256 tokens per core
HPC = H // NCORE          # 2 q-heads per core
DKT = D // P              # 16
IKT = I // P              # 8
IMT = I // P              # 8
DMT = D // P              # 16
NCH = S // 512            # 4 q-chunks
EPS = 1e-6
ISQ_DH = float(1.0 / np.sqrt(DH))


# ======================================================================
# device program
# ======================================================================

def build_module(dbg=False):
    nc = bacc.Bacc("TRN2", target_bir_lowering=False, debug=False, num_devices=NCORE)

    def inp(name, shape, dt):
        return nc.dram_tensor(name, list(shape), dt, kind="ExternalInput")

    x_slT = inp("x_slT", [P, DKT, TPC], F32)         # own slice, pre-transposed
    xT_full = inp("xT_full", [P, DKT, S], F32)       # full x, pre-transposed
    wqkv_h = inp("wqkv_h", [P, DKT, 512], F16)       # x64-scaled fp16 hi
    wqkv_l = inp("wqkv_l", [P, DKT, 512], F16)       # x64-scaled fp16 residual
    wo_h = inp("wo_h", [P, DMT, DKT, P], F16)
    wo_l = inp("wo_l", [P, DMT, DKT, P], F16)
    rw_h = inp("rw_h", [P, DKT, E], F16)
    rw_l = inp("rw_l", [P, DKT, E], F16)
    ones16_in = inp("ones16_in", [P, 1], F16)
    rbias = inp("rbias", [E, 1], F32)
    cos_in = inp("cos_in", [RD, S], F32)
    sin_in = inp("sin_in", [RD, S], F32)
    id_f = inp("id_f", [P, P], F32)
    ones_in = inp("ones_in", [P, 1], F32R)
    qnw_in = inp("qnw_in", [P, HPC], F32)
    knw_in = inp("knw_in", [P, 1], F32)
    mask_in = inp("mask_in", [P, 4, 512], F32)
    wg_p = inp("wg_p", [P, E, IMT, DKT, P], BF16)
    wu_p = inp("wu_p", [P, E, IMT, DKT, P], BF16)
    wd_p = inp("wd_p", [P, E, DMT, IKT, P], BF16)

    out_sl = nc.dram_tensor("out_sl", [TPC, D], F32, kind="ExternalOutput")
    dbg_t = {}
    if dbg:
        for nm, shp in [("d_sown", [1, TPC]), ("d_qT", [P, HPC, S]),
                        ("d_kT", [P, S]), ("d_cq", [1, S]), ("d_ck", [1, S]),
                        ("d_oT", [P, HPC, S]), ("d_x1T", [P, DMT, TPC]),
                        ("d_cmb", [E, TPC]), ("d_s2", [1, TPC])]:
            dbg_t[nm] = nc.dram_tensor(nm, shp, F32, kind="ExternalOutput")

    with tile.TileContext(nc) as tc, contextlib.ExitStack() as ctx:
        persist = ctx.enter_context(tc.tile_pool(name="persist", bufs=1))
        dram = ctx.enter_context(tc.tile_pool(name="dram", bufs=1, space="DRAM"))

        # ---------- persistent constants / long-lived small tiles ----------
        ones_sb = persist.tile([P, 1], F32R, tag="ones_sb")
        nc.sync.dma_start(ones_sb[:], ones_in.ap())
        ones16_sb = persist.tile([P, 1], F16, tag="ones16_sb")
        nc.sync.dma_start(ones16_sb[:], ones16_in.ap())
        idf_sb = persist.tile([P, P], F32, tag="idf_sb")
        nc.sync.dma_start(idf_sb[:], id_f.ap())
        qnw_sb = persist.tile([P, HPC], F32, tag="qnw_sb")
        nc.sync.dma_start(qnw_sb[:], qnw_in.ap())
        knw_sb = persist.tile([P, 1], F32, tag="knw_sb")
        nc.sync.dma_start(knw_sb[:], knw_in.ap())
        rb_sb = persist.tile([E, 1], F32, tag="rb_sb")
        nc.sync.dma_start(rb_sb[:], rbias.ap())
        xT_own = persist.tile([P, DKT, TPC], F32, tag="xT_own")
        nc.sync.dma_start(xT_own[:], x_slT.ap())
        x1T = persist.tile([P, DMT, TPC], F32, tag="x1T")
        h2b = persist.tile([P, DKT, TPC], BF16, tag="h2b")
        cmb_row = persist.tile([1, E, TPC], BF16, tag="cmb_row")

        def newton_rsqrt(row_ap, tmp_pool, n):
            """row <- refined 1/sqrt(u) given row holding u (in fp32)."""
            u = tmp_pool.tile([1, n], F32, tag="nr_u")
            nc.vector.tensor_copy(u[:], row_ap)
            nc.scalar.activation(row_ap, row_ap, AF.Sqrt)
            nc.vector.reciprocal(row_ap, row_ap)
            t = tmp_pool.tile([1, n], F32, tag="nr_t")
            nc.vector.tensor_mul(t[:], u[:], row_ap)
            nc.vector.tensor_mul(t[:], t[:], row_ap)
            nc.vector.tensor_scalar(t[:], t[:], -0.5, 1.5, OP.mult, OP.add)
            nc.vector.tensor_mul(row_ap, row_ap, t[:])

        def sumsq2(dst_ps, src_ap, pool, shape, first, last, from_psum=False):
            """dst_ps[1,n] (+)= column sums of src^2, hi/lo compensated."""
            p_, n_ = shape
            sqf = pool.tile([p_, n_], F32, tag="ss_f")
            nc.scalar.activation(sqf[:], src_ap, AF.Square)
            sqh = pool.tile([p_, n_], F32R, tag="ss_h")
            nc.vector.tensor_copy(sqh[:], sqf[:])
            sql = pool.tile([p_, n_], F32R, tag="ss_l")
            nc.vector.tensor_sub(sql[:], sqf[:], sqh[:])
            nc.tensor.matmul(dst_ps[:], ones_sb[:], sqh[:], start=first, stop=False)
            nc.tensor.matmul(dst_ps[:], ones_sb[:], sql[:], start=False, stop=last)

        with (
            tc.tile_pool(name="pAtt", bufs=1) as pAtt,   # q/k splits, v_tm: ph3-6
        ):
            mask_sb = pAtt.tile([P, 4, 512], F32, tag="mask_sb")
            nc.sync.dma_start(mask_sb[:], mask_in.ap())

            # x is replicated as a full input on every core: no AllGather.
            # The ln1-rms sumsq row for ALL tokens is accumulated from the
            # same rhs tiles the qkv loop streams through SBUF.
            s_row = pAtt.tile([1, S], F32, tag="s_row")

            # ---------- phase 3: fused QKV projection (3-pass compensated) ----------
            qT = pAtt.tile([P, HPC, S], F32, tag="qT")
            kT = pAtt.tile([P, S], F32, tag="kT")
            with tc.tile_pool(name="pV", bufs=1) as pV:
                vT = pV.tile([P, S], F32, tag="vT")
                arin_q = pV.tile([1, S], F32, tag="arin_q")
                arin_k = pV.tile([1, S], F32, tag="arin_k")
                with (
                    tc.tile_pool(name="qkv_w", bufs=3) as qkv_w,
                    tc.tile_pool(name="qkv_rhs", bufs=3) as rhs_pool,
                    tc.tile_pool(name="qkv_sq", bufs=3) as qsq_pool,
                    tc.tile_pool(name="qkv_ps", bufs=1, space="PSUM") as qkv_ps,
                    tc.tile_pool(name="qs_ps", bufs=1, space="PSUM") as qs_ps,
                ):
                    for ch in range(NCH):
                        pts = [qkv_ps.tile([P, 512], F32, tag=f"qkvp{m}",
                                           name=f"qkvp{m}") for m in range(4)]
                        sacc = qs_ps.tile([1, 512], F32, tag="sacc")
                        for kd in range(DKT):
                            wh_t = qkv_w.tile([P, 512], F16, tag="wh_t")
                            nc.sync.dma_start(wh_t[:], wqkv_h.ap()[:, kd])
                            wl_t = qkv_w.tile([P, 512], F16, tag="wl_t")
                            nc.sync.dma_start(wl_t[:], wqkv_l.ap()[:, kd])
                            rhs = rhs_pool.tile([P, 512], F32, tag="rhs")
                            nc.sync.dma_start(rhs[:],
                                              xT_full.ap()[:, kd,
                                                           ch * 512:(ch + 1) * 512])
                            rhs2 = rhs[:]
                            sumsq2(sacc, rhs2, qsq_pool, (P, 512),
                                   first=(kd == 0), last=(kd == DKT - 1))
                            rh = rhs_pool.tile([P, 512], F16, tag="rh")
                            nc.vector.tensor_copy(rh[:], rhs2)
                            rl = rhs_pool.tile([P, 512], F16, tag="rl")
                            nc.vector.tensor_sub(rl[:], rhs2, rh[:])
                            for m in range(4):
                                cs_w = slice(m * P, (m + 1) * P)
                                nc.tensor.matmul(pts[m][:], wh_t[:, cs_w], rh[:],
                                                 start=(kd == 0), stop=False)
                                nc.tensor.matmul(pts[m][:], wh_t[:, cs_w], rl[:],
                                                 start=False, stop=False)
                                nc.tensor.matmul(pts[m][:], wl_t[:, cs_w], rh[:],
                                                 start=False,
                                                 stop=(kd == DKT - 1))
                        cs = slice(ch * 512, (ch + 1) * 512)
                        qacc = qs_ps.tile([1, 512], F32, tag="qacc")
                        kacc = qs_ps.tile([1, 512], F32, tag="kacc")
                        for m in range(2):
                            nc.vector.tensor_scalar_mul(qT[:, m, cs], pts[m][:],
                                                        1.0 / 64.0)
                            sumsq2(qacc, qT[:, m, cs], qsq_pool, (P, 512),
                                   first=(m == 0), last=(m == 1))
                        nc.vector.tensor_scalar_mul(kT[:, cs], pts[2][:], 1.0 / 64.0)
                        sumsq2(kacc, kT[:, cs], qsq_pool, (P, 512),
                               first=True, last=True)
                        nc.vector.tensor_scalar_mul(vT[:, cs], pts[3][:], 1.0 / 64.0)
                        nc.vector.tensor_copy(arin_q[:, cs], qacc[:])
                        nc.vector.tensor_copy(arin_k[:, cs], kacc[:])
                        nc.vector.tensor_scalar(s_row[:, cs], sacc[:], 1.0 / D, EPS,
                                                OP.mult, OP.add)

                # ---------- phase 4: AllReduce qk sumsq; scale rows ----------
                ar_in = dram.tile([2, S], F32, tag="ar_in")
                ar_out = dram.tile([2, S], F32, addr_space="Shared", tag="ar_out")
                nc.sync.dma_start(ar_in[0:1, :], arin_q[:])
                nc.sync.dma_start(ar_in[1:2, :], arin_k[:])
                nc.gpsimd.collective_compute("AllReduce", OP.add, replica_groups=RG8,
                                             ins=[ar_in.opt()], outs=[ar_out.opt()])
                with tc.tile_pool(name="p4", bufs=1) as p4:
                    cq = p4.tile([1, S], F32, tag="cq")
                    ck = p4.tile([1, S], F32, tag="ck")
                    with (
                        tc.tile_pool(name="p4r", bufs=1) as p4r,
                        tc.tile_pool(name="crow", bufs=1) as crow,
                    ):
                        arq_r = p4r.tile([1, S], F32, tag="arq_r")
                        nc.sync.dma_start(arq_r[:], ar_out[0:1, :])
                        ark_r = p4r.tile([1, S], F32, tag="ark_r")
                        nc.sync.dma_start(ark_r[:], ar_out[1:2, :])
                        newton_rsqrt(s_row[:], p4r, S)
                        sflat = s_row[:]
                        for (dst, arrow, mean_div, post) in (
                                (cq, arq_r, float(H * DH), ISQ_DH),
                                (ck, ark_r, float(2 * KVH * DH), 1.0)):
                            t1 = crow.tile([1, S], F32, tag="t1")
                            nc.vector.tensor_mul(t1[:], sflat, sflat)
                            nc.vector.tensor_mul(t1[:], t1[:], arrow[:])
                            nc.vector.tensor_scalar(t1[:], t1[:], 1.0 / mean_div, EPS,
                                                    OP.mult, OP.add)
                            newton_rsqrt(t1[:], crow, S)
                            nc.vector.tensor_mul(t1[:], t1[:], sflat)
                            nc.vector.tensor_scalar_mul(dst[:], t1[:], post)
                        # v has no qk-norm to cancel the ln1 rms: apply the
                        # per-token scale row to vT directly
                        bsv = crow.tile([P, S], F32, tag="bsv")
                        nc.gpsimd.partition_broadcast(bsv[:], sflat)
                        nc.vector.tensor_mul(vT[:], vT[:], bsv[:])
                    if dbg:
                        nc.gpsimd.dma_start(dbg_t["d_cq"].ap(), cq[:])
                        nc.gpsimd.dma_start(dbg_t["d_ck"].ap(), ck[:])

                    # ---------- phase 5: qk norms + rope (fp32); splits ----------
                    with (
                        tc.tile_pool(name="bc", bufs=1) as bcp,
                        tc.tile_pool(name="rope", bufs=1) as rp,
                    ):
                        cos_sb = rp.tile([RD, S], F32, tag="cos_sb")
                        nc.sync.dma_start(cos_sb[:], cos_in.ap())
                        sin_sb = rp.tile([RD, S], F32, tag="sin_sb")
                        nc.sync.dma_start(sin_sb[:], sin_in.ap())
                        bq = bcp.tile([P, S], F32, tag="bq")
                        nc.gpsimd.partition_broadcast(bq[:], cq[:])
                        bk = bcp.tile([P, S], F32, tag="bk")
                        nc.gpsimd.partition_broadcast(bk[:], ck[:])
                        for m in range(HPC):
                            nc.vector.tensor_mul(qT[:, m, :], qT[:, m, :], bq[:])
                            nc.vector.tensor_scalar_mul(qT[:, m, :], qT[:, m, :],
                                                        qnw_sb[:, m:m + 1])
                        nc.vector.tensor_mul(kT[:], kT[:], bk[:])
                        nc.vector.tensor_scalar_mul(kT[:], kT[:], knw_sb[:, 0:1])
                        HF = RD // 2
                        for ap_ in [qT[:, 0, :], qT[:, 1, :], kT[:]]:
                            # qsh = rotate_half layout: [q[HF:RD]; q[0:HF]]
                            qsh = rp.tile([RD, S], F32, tag="qsh")
                            nc.sync.dma_start(qsh[0:HF, :], ap_[HF:RD, :])
                            nc.sync.dma_start(qsh[HF:RD, :], ap_[0:HF, :])
                            # sin table has rows 0:HF pre-negated on host
                            nc.vector.tensor_mul(qsh[0:RD, :], qsh[0:RD, :],
                                                 sin_sb[0:RD, :])
                            nc.vector.tensor_mul(ap_[0:RD, :], ap_[0:RD, :],
                                                 cos_sb[0:RD, :])
                            nc.vector.tensor_add(ap_[0:RD, :], ap_[0:RD, :],
                                                 qsh[0:RD, :])

                if dbg:
                    nc.gpsimd.dma_start(dbg_t["d_qT"].ap(), qT[:])
                    nc.gpsimd.dma_start(dbg_t["d_kT"].ap(), kT[:])

                # fp16 hi/lo splits of roped k
                kh16 = pAtt.tile([P, S], F16, tag="kh16")
                kl16 = pAtt.tile([P, S], F16, tag="kl16")
                nc.vector.tensor_copy(kh16[:], kT[:])
                nc.vector.tensor_sub(kl16[:], kT[:], kh16[:])

                # v to token-major (fp16 hi/lo)
                vh16 = pAtt.tile([P, DKT, DH], F16, tag="vh16")
                vl16 = pAtt.tile([P, DKT, DH], F16, tag="vl16")
                with (
                    tc.tile_pool(name="vt_ps", bufs=2, space="PSUM") as vt_ps,
                    tc.tile_pool(name="vt_sb", bufs=2) as vt_sb,
                ):
                    for kt in range(DKT):
                        pt = vt_ps.tile([P, P], F32, tag="vt")
                        nc.tensor.transpose(pt[:], vT[:, kt * P:(kt + 1) * P],
                                            idf_sb[:])
                        nc.vector.tensor_copy(vh16[:, kt, :], pt[:])
                        nc.vector.tensor_sub(vl16[:, kt, :], pt[:], vh16[:, kt, :])

            # ---------- phase 6: attention (compensated scores + av) ----------
            with tc.tile_pool(name="pO", bufs=1) as pO:
                oT = pO.tile([P, HPC, S], F32, tag="oT")
                with (
                    tc.tile_pool(name="sc_ps", bufs=2, space="PSUM") as sc_ps,
                    tc.tile_pool(name="o_ps", bufs=2, space="PSUM") as o_ps,
                    tc.tile_pool(name="sm_ps", bufs=2, space="PSUM") as sm_ps,
                    tc.tile_pool(name="eT", bufs=3) as e_pool,
                    tc.tile_pool(name="att_sb", bufs=2) as att_sb,
                ):
                    for m in range(HPC):
                        for qc in range(NCH):
                            nkt = 4 * qc + 4
                            qsl = slice(qc * 512, (qc + 1) * 512)
                            qh = att_sb.tile([P, 512], F16, tag="qh")
                            nc.vector.tensor_copy(qh[:], qT[:, m, qsl])
                            ql = att_sb.tile([P, 512], F16, tag="ql")
                            nc.vector.tensor_sub(ql[:], qT[:, m, qsl], qh[:])
                            opsum = o_ps.tile([P, 512], F32, tag="o")
                            spsum = sm_ps.tile([1, 512], F32, tag="s")
                            for kt in range(nkt):
                                ks = slice(kt * P, (kt + 1) * P)
                                scp = sc_ps.tile([P, 512], F32, tag="sc")
                                nc.tensor.matmul(scp[:], kh16[:, ks], qh[:],
                                                 start=True, stop=False)
                                nc.tensor.matmul(scp[:], kh16[:, ks], ql[:],
                                                 start=False, stop=False)
                                nc.tensor.matmul(scp[:], kl16[:, ks], qh[:],
                                                 start=False, stop=True)
                                eT = e_pool.tile([P, 512], F32, tag="e")
                                nc.scalar.activation(eT[:], scp[:], AF.Exp)
                                if kt >= 4 * qc:
                                    # mask table pre-scaled by 0.25 on host
                                    nc.vector.tensor_mul(eT[:], eT[:],
                                                         mask_sb[:, kt - 4 * qc, :])
                                else:
                                    nc.vector.tensor_scalar_mul(eT[:], eT[:], 0.25)
                                eh = e_pool.tile([P, 512], F16, tag="eh")
                                nc.vector.tensor_copy(eh[:], eT[:])
                                el = e_pool.tile([P, 512], F16, tag="el")
                                nc.vector.tensor_sub(el[:], eT[:], eh[:])
                                nc.tensor.matmul(spsum[:], ones16_sb[:], eh[:],
                                                 start=(kt == 0), stop=False)
                                nc.tensor.matmul(spsum[:], ones16_sb[:], el[:],
                                                 start=False, stop=(kt == nkt - 1))
                                nc.tensor.matmul(opsum[:], vh16[:, kt, :], eh[:],
                                                 start=(kt == 0), stop=False)
                                nc.tensor.matmul(opsum[:], vh16[:, kt, :], el[:],
                                                 start=False, stop=False)
                                nc.tensor.matmul(opsum[:], vl16[:, kt, :], eh[:],
                                                 start=False, stop=(kt == nkt - 1))
                            rrow = att_sb.tile([1, 512], F32, tag="rr")
                            nc.vector.reciprocal(rrow[:], spsum[:])
                            brr = att_sb.tile([P, 512], F32, tag="brr")
                            nc.gpsimd.partition_broadcast(brr[:], rrow[:])
                            nc.vector.tensor_mul(oT[:, m, qsl], opsum[:], brr[:])

                if dbg:
                    nc.gpsimd.dma_start(dbg_t["d_oT"].ap(), oT[:])
                # ---------- phase 7: AllToAll o^T -> token-sharded ----------
                a2a_in = dram.tile([NCORE, HPC * P, TPC], F32, tag="a2a_in")
                a2a_out = dram.tile([NCORE, HPC * P, TPC], F32, tag="a2a_out")
                for j in range(NCORE):
                    nc.sync.dma_start(a2a_in[j].rearrange("(m p) u -> p m u", p=P),
                                      oT[:, :, j * TPC:(j + 1) * TPC])
                nc.gpsimd.collective_compute("AllToAll", OP.bypass, replica_groups=RG8,
                                             ins=[a2a_in.opt()], outs=[a2a_out.opt()])
            oTo = a2a_out.rearrange("r q u -> (r q) u")   # [H*DH, TPC] global odim rows

            # ---------- phase 8: o-proj (3-pass) + residual -> x1 (fp32) ----------
            with (
                tc.tile_pool(name="p8", bufs=1) as p8,
                tc.tile_pool(name="wo_str", bufs=3) as wo_str,
                tc.tile_pool(name="op_ps", bufs=2, space="PSUM") as op_ps,
            ):
                oTo_f = p8.tile([P, DKT, TPC], F32, tag="oTo_f")
                nc.sync.dma_start(oTo_f[:], oTo.rearrange("(ko p) u -> p ko u", p=P))
                oh16 = p8.tile([P, DKT, TPC], F16, tag="oh16")
                ol16 = p8.tile([P, DKT, TPC], F16, tag="ol16")
                for ko in range(DKT):
                    nc.vector.tensor_copy(oh16[:, ko, :], oTo_f[:, ko, :])
                    nc.vector.tensor_sub(ol16[:, ko, :], oTo_f[:, ko, :],
                                         oh16[:, ko, :])
                for md in range(DMT):
                    wth = wo_str.tile([P, DKT, P], F16, tag="wth")
                    nc.sync.dma_start(wth[:], wo_h.ap()[:, md])
                    wtl = wo_str.tile([P, DKT, P], F16, tag="wtl")
                    nc.sync.dma_start(wtl[:], wo_l.ap()[:, md])
                    pt = op_ps.tile([P, TPC], F32, tag="op")
                    for ko in range(DKT):
                        nc.tensor.matmul(pt[:], wth[:, ko, :], oh16[:, ko, :],
                                         start=(ko == 0), stop=False)
                        nc.tensor.matmul(pt[:], wth[:, ko, :], ol16[:, ko, :],
                                         start=False, stop=False)
                        nc.tensor.matmul(pt[:], wtl[:, ko, :], oh16[:, ko, :],
                                         start=False, stop=(ko == DKT - 1))
                    nc.vector.scalar_tensor_tensor(x1T[:, md, :], pt[:], 1.0 / 64.0,
                                                   xT_own[:, md, :], OP.mult, OP.add)

        if dbg:
            nc.gpsimd.dma_start(dbg_t["d_x1T"].ap(), x1T[:])
        # ---------- phase 9: ln2 rms, h2, router, combine ----------
        with (
            tc.tile_pool(name="p9", bufs=1) as p9,
            tc.tile_pool(name="s2_sb", bufs=3) as s2_pool,
            tc.tile_pool(name="s2_ps", bufs=1, space="PSUM") as s2_ps,
            tc.tile_pool(name="rt_sb", bufs=1) as rt_sb,
            tc.tile_pool(name="rt_ps", bufs=1, space="PSUM") as rt_ps,
        ):
            s2row = p9.tile([1, TPC], F32, tag="s2row")
            acc2 = s2_ps.tile([1, TPC], F32, tag="acc2")
            for kd in range(DKT):
                sumsq2(acc2, x1T[:, kd, :], s2_pool, (P, TPC),
                       first=(kd == 0), last=(kd == DKT - 1))
            nc.vector.tensor_scalar(s2row[:], acc2[:], 1.0 / D, EPS, OP.mult, OP.add)
            newton_rsqrt(s2row[:], s2_pool, TPC)

            h2f = p9.tile([P, DKT, TPC], F32, tag="h2f")
            bs2 = p9.tile([P, TPC], F32, tag="bs2")
            nc.gpsimd.partition_broadcast(bs2[:], s2row[:])
            for kd in range(DKT):
                nc.vector.tensor_mul(h2f[:, kd, :], x1T[:, kd, :], bs2[:])
                nc.vector.tensor_copy(h2b[:, kd, :], h2f[:, kd, :])
            h2h16 = p9.tile([P, DKT, TPC], F16, tag="h2h16")
            h2l16 = p9.tile([P, DKT, TPC], F16, tag="h2l16")
            for kd in range(DKT):
                nc.vector.tensor_copy(h2h16[:, kd, :], h2f[:, kd, :])
                nc.vector.tensor_sub(h2l16[:, kd, :], h2f[:, kd, :],
                                     h2h16[:, kd, :])
            rwh_sb = rt_sb.tile([P, DKT, E], F16, tag="rwh_sb")
            nc.sync.dma_start(rwh_sb[:], rw_h.ap())
            rwl_sb = rt_sb.tile([P, DKT, E], F16, tag="rwl_sb")
            nc.sync.dma_start(rwl_sb[:], rw_l.ap())
            lg = rt_ps.tile([E, TPC], F32, tag="lg")
            for kd in range(DKT):
                nc.tensor.matmul(lg[:], rwh_sb[:, kd, :], h2h16[:, kd, :],
                                 start=(kd == 0), stop=False)
                nc.tensor.matmul(lg[:], rwh_sb[:, kd, :], h2l16[:, kd, :],
                                 start=False, stop=False)
                nc.tensor.matmul(lg[:], rwl_sb[:, kd, :], h2h16[:, kd, :],
                                 start=False, stop=(kd == DKT - 1))
            lgf = rt_sb.tile([E, TPC], F32, tag="lgf")
            nc.vector.tensor_scalar_mul(lgf[:], lg[:], 1.0 / 64.0)
            sg = rt_sb.tile([E, TPC], F32, tag="sg")
            nc.scalar.activation(sg[:], lgf[:], AF.Sigmoid)
            biased = rt_sb.tile([E, TPC], F32, tag="biased")
            nc.vector.tensor_scalar_add(biased[:], sg[:], rb_sb[:, 0:1])
            m1 = rt_sb.tile([E, TPC], F32, tag="m1")
            nc.gpsimd.partition_all_reduce(m1[:], biased[:], channels=E,
                                           reduce_op=bass_isa.ReduceOp.max)
            eq = rt_sb.tile([E, TPC], F32, tag="eq")
            nc.vector.tensor_tensor(eq[:], biased[:], m1[:], OP.is_equal)
            nc.vector.tensor_scalar_mul(eq[:], eq[:], -1e9)
            nc.vector.tensor_add(eq[:], eq[:], biased[:])
            m2 = rt_sb.tile([E, TPC], F32, tag="m2")
            nc.gpsimd.partition_all_reduce(m2[:], eq[:], channels=E,
                                           reduce_op=bass_isa.ReduceOp.max)
            ind = rt_sb.tile([E, TPC], F32, tag="ind")
            nc.vector.tensor_tensor(ind[:], biased[:], m2[:], OP.is_ge)
            aff = rt_sb.tile([E, TPC], F32, tag="aff")
            nc.vector.tensor_mul(aff[:], sg[:], ind[:])
            den = rt_sb.tile([E, TPC], F32, tag="den")
            nc.gpsimd.partition_all_reduce(den[:], aff[:], channels=E,
                                           reduce_op=bass_isa.ReduceOp.add)
            rden = rt_sb.tile([E, TPC], F32, tag="rden")
            nc.vector.reciprocal(rden[:], den[:])
            nc.vector.tensor_mul(aff[:], aff[:], rden[:])
            cmb_bf = rt_sb.tile([E, TPC], BF16, tag="cmb_bf")
            nc.vector.tensor_copy(cmb_bf[:], aff[:])
            nc.sync.dma_start(cmb_row[:], cmb_bf[:])
            if dbg:
                nc.gpsimd.dma_start(dbg_t["d_cmb"].ap(), aff[:])
                nc.gpsimd.dma_start(dbg_t["d_s2"].ap(), s2row[:])

        # ---------- phase 10: dense MoE over all experts (bf16) ----------
        with (
            tc.tile_pool(name="p10", bufs=1) as p10,
            tc.tile_pool(name="wmoe", bufs=2) as wmoe,
            tc.tile_pool(name="moe_ps", bufs=2, space="PSUM") as moe_ps,
            tc.tile_pool(name="moe_sb", bufs=2) as moe_sb,
        ):
            act_all = p10.tile([P, E, IMT, TPC], BF16, tag="act_all")
            up_bf = p10.tile([P, IMT, TPC], BF16, tag="up_bf")
            out_fm = p10.tile([P, DMT, TPC], F32, tag="out_fm")
            for e in range(E):
                bce = moe_sb.tile([P, TPC], BF16, tag="bce")
                nc.gpsimd.partition_broadcast(bce[:], cmb_row[0:1, e, :])
                for mi in range(IMT):
                    wt = wmoe.tile([P, DKT, P], BF16, tag="wmu")
                    nc.gpsimd.dma_start(wt[:], wu_p.ap()[:, e, mi])
                    pt = moe_ps.tile([P, TPC], F32, tag="up")
                    for kd in range(DKT):
                        nc.tensor.matmul(pt[:], wt[:, kd, :], h2b[:, kd, :],
                                         start=(kd == 0), stop=(kd == DKT - 1))
                    nc.vector.tensor_copy(up_bf[:, mi, :], pt[:])
                for mi in range(IMT):
                    wt = wmoe.tile([P, DKT, P], BF16, tag="wmg")
                    nc.gpsimd.dma_start(wt[:], wg_p.ap()[:, e, mi])
                    pt = moe_ps.tile([P, TPC], F32, tag="gate")
                    for kd in range(DKT):
                        nc.tensor.matmul(pt[:], wt[:, kd, :], h2b[:, kd, :],
                                         start=(kd == 0), stop=(kd == DKT - 1))
                    gs = moe_sb.tile([P, TPC], BF16, tag="gs")
                    nc.scalar.activation(gs[:], pt[:], AF.Silu)
                    nc.vector.tensor_mul(gs[:], gs[:], up_bf[:, mi, :])
                    nc.vector.tensor_mul(act_all[:, e, mi, :], gs[:], bce[:])
            for md in range(DMT):
                pt = moe_ps.tile([P, TPC], F32, tag="dn")
                for e in range(E):
                    wt = wmoe.tile([P, IKT, P], BF16, tag="wmd")
                    nc.gpsimd.dma_start(wt[:], wd_p.ap()[:, e, md])
                    for ki in range(IKT):
                        nc.tensor.matmul(pt[:], wt[:, ki, :], act_all[:, e, ki, :],
                                         start=(e == 0 and ki == 0),
                                         stop=(e == E - 1 and ki == IKT - 1))
                nc.vector.tensor_add(out_fm[:, md, :], pt[:], x1T[:, md, :])

            # ---------- phase 11: transpose to token-major; write output ----------
            out_tm = p10.tile([P, 2, DMT, P], F32, tag="out_tm")
            with tc.tile_pool(name="ot_ps", bufs=2, space="PSUM") as ot_ps:
                for md in range(DMT):
                    for tb in range(2):
                        pt = ot_ps.tile([P, P], F32, tag="ot")
                        nc.tensor.transpose(pt[:], out_fm[:, md, tb * P:(tb + 1) * P],
                                            idf_sb[:])
                        nc.vector.tensor_copy(out_tm[:, tb, md, :], pt[:])
            nc.sync.dma_start(
                out_sl.ap().rearrange("(tb p) (md c) -> p tb md c", p=P, c=P),
                out_tm[:])

    nc.compile()
    return nc


# ======================================================================
# host-side input preparation
# ======================================================================

def _trunc_hi(w, bits=12):
    """Zero all but the top `bits` mantissa bits (hi half survives fp32r rounding)."""
    u = np.ascontiguousarray(w, dtype=np.float32).view(np.uint32)
    mask = np.uint32(0xFFFFFFFF) << np.uint32(23 - bits)
    return (u & mask).view(np.float32)


def prep_in_maps(inputs):
    f32 = lambda a: np.ascontiguousarray(np.asarray(a), dtype=np.float32)
    x = f32(inputs["x"]).reshape(S, D)
    ln1 = f32(inputs["ln1_w"])
    ln2 = f32(inputs["ln2_w"])
    wq = f32(inputs["wq"]) * ln1[:, None]
    wk = f32(inputs["wk"]) * ln1[:, None]
    wv = f32(inputs["wv"]) * ln1[:, None]
    wo = f32(inputs["wo"])
    qnw = f32(inputs["qnorm_w"])
    knw = f32(inputs["knorm_w"])
    rw = f32(inputs["router_w"]) * ln2[:, None]
    rb = f32(inputs["router_bias"]).reshape(E, 1)
    wg = f32(inputs["wg"]) * ln2[None, :, None]
    wu = f32(inputs["wu"]) * ln2[None, :, None]
    wd = f32(inputs["wd"])

    # The rope tables must BIT-MATCH the fp32 jax reference: at angles of
    # ~2000 rad a 1-ulp difference in inv_freq moves cos/sin by ~1e-4, which
    # perturbs x1 enough to flip near-tie router decisions.  So compute them
    # with jax itself on the CPU backend, mirroring the reference ops.
    import jax as _jax
    import jax.numpy as _jnp
    with _jax.default_device(_jax.devices("cpu")[0]):
        _pos = _jnp.arange(S, dtype=_jnp.float32)
        _invf = 1.0 / (1000000.0 ** (_jnp.arange(0, RD, 2, dtype=_jnp.float32) / RD))
        _ang = _pos[:, None] * _invf[None, :]
        _emb = _jnp.concatenate([_ang, _ang], axis=-1)       # [S, RD]
        cos_t = np.ascontiguousarray(np.asarray(_jnp.cos(_emb)).T)  # [RD, S]
        sin_t = np.ascontiguousarray(np.asarray(_jnp.sin(_emb)).T)
    sin_t[:RD // 2] *= -1.0   # fold rotate_half sign into the table

    ident = np.eye(P, dtype=np.float32)
    ones_c = np.ones((P, 1), dtype=np.float32)
    p_i = np.arange(P)[:, None, None]
    off_i = np.arange(4)[None, :, None]
    q_i = np.arange(512)[None, None, :]
    mask = ((P * off_i + p_i) <= q_i).astype(np.float32) * 0.25

    pack_kd = lambda w: np.ascontiguousarray(
        w.reshape(DKT, P, w.shape[1]).transpose(1, 0, 2))   # [D, C] -> [P, DKT, C]

    def f16_split(w):
        ws = (w * 64.0).astype(np.float32)
        hi = ws.astype(np.float16)
        lo = (ws - hi.astype(np.float32)).astype(np.float16)
        return hi, lo

    bf = ml_dtypes.bfloat16
    wg_pk = np.ascontiguousarray(
        wg.reshape(E, DKT, P, IMT, P).transpose(2, 0, 3, 1, 4).astype(bf))
    wu_pk = np.ascontiguousarray(
        wu.reshape(E, DKT, P, IMT, P).transpose(2, 0, 3, 1, 4).astype(bf))
    wd_pk = np.ascontiguousarray(
        wd.reshape(E, IKT, P, DMT, P).transpose(2, 0, 3, 1, 4).astype(bf))
    wo_hi, wo_lo = f16_split(wo)
    pack_wo = lambda w: np.ascontiguousarray(
        w.reshape(DKT, P, DMT, P).transpose(1, 2, 0, 3))
    wo_hp = pack_wo(wo_hi)
    wo_lp = pack_wo(wo_lo)
    rw_hi, rw_lo = f16_split(rw)

    in_maps = []
    for c in range(NCORE):
        qcols = slice(c * HPC * DH, (c + 1) * HPC * DH)
        kvcols = slice((c // 2) * DH, (c // 2 + 1) * DH)
        wqkv_c = np.concatenate([wq[:, qcols], wk[:, kvcols], wv[:, kvcols]], axis=1)
        wqkv_hi, wqkv_lo = f16_split(wqkv_c)
        qnw_c = np.ascontiguousarray(qnw[qcols].reshape(HPC, P).T)
        knw_c = np.ascontiguousarray(knw[kvcols].reshape(1, P).T)
        x_sl = x[c * TPC:(c + 1) * TPC]                      # [TPC, D]
        x_slT_c = np.ascontiguousarray(
            x_sl.T.reshape(DKT, P, TPC).transpose(1, 0, 2))  # [P, DKT, TPC]
        if c == 0:
            xT_full_c = np.ascontiguousarray(
                x.T.reshape(DKT, P, S).transpose(1, 0, 2))   # [P, DKT, S]
        in_maps.append({
            "x_slT": x_slT_c,
            "xT_full": xT_full_c,
            "wqkv_h": pack_kd(wqkv_hi),
            "wqkv_l": pack_kd(wqkv_lo),
            "wo_h": wo_hp,
            "wo_l": wo_lp,
            "rw_h": pack_kd(rw_hi),
            "rw_l": pack_kd(rw_lo),
            "ones16_in": ones_c.astype(np.float16),
            "rbias": rb,
            "cos_in": cos_t,
            "sin_in": sin_t,
            "id_f": ident,
            "ones_in": ones_c,
            "qnw_in": qnw_c,
            "knw_in": knw_c,
            "mask_in": mask,
            "wg_p": wg_pk,
            "wu_p": wu_pk,
            "wd_p": wd_pk,
        })
    return in_maps


_CACHE = {}


def get_module():
    if "nc" not in _CACHE:
        _CACHE["nc"] = build_module()
    return _CACHE["nc"]


def kernel(**inputs) -> np.ndarray:
    nc = get_module()
    in_maps = prep_in_maps(inputs)
    res = bass_utils.run_bass_kernel_spmd(nc, in_maps, core_ids=list(range(NCORE)))
    out = np.concatenate([res.results[c]["out_sl"] for c in range(NCORE)], axis=0)
    return out.reshape(1, S, D).astype(np.float32)


if __name__ == "__main__":
    build_module()
    print("module built ok")


# revision 23
# speedup vs baseline: 1.1427x; 1.1427x over previous
"""MiniMax-M2 decoder layer (attention + sigmoid-router top-2 MoE) on 8 TRN2 NeuronCores.

Sharding: head-parallel attention (2 q-heads + 1 kv-head per core), token-parallel
everything else (256 tokens per core).  x is replicated to every core as a
pre-transposed input (no AllGather); the ln1-rms row for all tokens is
accumulated from the same tiles the qkv loop streams.  Collectives: AllReduce
of qk-norm sumsq partials, AllToAll to reshard o^T from head- to
token-sharded.  Dense MoE over all 8 experts per token slice with bf16 weights
(combine weights zero unrouted experts).  The v projection gets the ln1-rms
row applied explicitly (q/k absorb it via the qk-norm scale rows).

Precision: the router's top-2 selection has tie gaps down to ~1.5e-5 in
sigmoid space, and the TRN2 PE rounds every fp32/fp32r product to ~12-15
mantissa bits, so no fp32-mode GEMM is accurate enough.  Everything feeding
x1 (qkv, scores, attn*v, o-proj, router) instead runs as hi/lo-compensated
FP16 matmuls (3-pass: wh*xh + wh*xl + wl*xh; fp16 products are exact in the
fp32 PSUM, giving ~2^-22 results at 3 cyc/row).  Weights are pre-scaled x64
on the host so fp16 residuals stay out of the subnormal range (unscaled on
PSUM evacuation); softmax numerators are scaled by 0.25 to stay under fp16
max (cancels in the ratio).  Row reductions (rmsnorm sumsq, softmax
denominators) use the ones-lhsT fp32r trick (values pass through the PE
exactly), and rsqrt rows get one Newton step.  RoPE tables are computed with
jax on CPU so they BIT-match the fp32 reference (a 1-ulp inv_freq difference
moves cos/sin by ~1e-4 at ~2000 rad and flips near-tie routing).  The MoE
itself stays bf16 (output tolerance 2e-2).

kernel(**inputs) takes full unsharded inputs, returns the full [1, S, D] output.
"""

import contextlib

import numpy as np
import ml_dtypes

import concourse.bass as bass
import concourse.mybir as mybir
import concourse.tile as tile
from concourse import bacc, bass_isa, bass_utils

F32 = mybir.dt.float32
F32R = mybir.dt.float32r
F16 = mybir.dt.float16
BF16 = mybir.dt.bfloat16
AF = mybir.ActivationFunctionType
OP = mybir.AluOpType
RG8 = [list(range(8))]

P = 128
D = 2048
H = 16
KVH = 4
DH = 128
RD = 64
E = 8
I = 1024
S = 2048
NCORE = 8
TPC = S // NCORE          # 256 tokens per core
HPC = H // NCORE          # 2 q-heads per core
DKT = D // P              # 16
IKT = I // P              # 8
IMT = I // P              # 8
DMT = D // P              # 16
NCH = S // 512            # 4 q-chunks
EPS = 1e-6
ISQ_DH = float(1.0 / np.sqrt(DH))


# ======================================================================
# device program
# ======================================================================

def build_module(dbg=False):
    nc = bacc.Bacc("TRN2", target_bir_lowering=False, debug=False, num_devices=NCORE)

    def inp(name, shape, dt):
        return nc.dram_tensor(name, list(shape), dt, kind="ExternalInput")

    x_slT = inp("x_slT", [P, DKT, TPC], F32)         # own slice, pre-transposed
    xT_full = inp("xT_full", [P, DKT, S], F32)       # full x, pre-transposed
    wqkv_h = inp("wqkv_h", [P, DKT, 512], F16)       # x64-scaled fp16 hi
    wqkv_l = inp("wqkv_l", [P, DKT, 512], F16)       # x64-scaled fp16 residual
    wo_h = inp("wo_h", [P, DMT, DKT, P], F16)
    wo_l = inp("wo_l", [P, DMT, DKT, P], F16)
    rw_h = inp("rw_h", [P, DKT, E], F16)
    rw_l = inp("rw_l", [P, DKT, E], F16)
    ones16_in = inp("ones16_in", [P, 1], F16)
    rbias = inp("rbias", [E, 1], F32)
    cos_in = inp("cos_in", [RD, S], F32)
    sin_in = inp("sin_in", [RD, S], F32)
    id_f = inp("id_f", [P, P], F32)
    ones_in = inp("ones_in", [P, 1], F32R)
    qnw_in = inp("qnw_in", [P, HPC], F32)
    knw_in = inp("knw_in", [P, 1], F32)
    mask_in = inp("mask_in", [P, 4, 512], F32)
    wg_p = inp("wg_p", [P, E, IMT, DKT, P], BF16)
    wu_p = inp("wu_p", [P, E, IMT, DKT, P], BF16)
    wd_p = inp("wd_p", [P, E, DMT, IKT, P], BF16)

    out_sl = nc.dram_tensor("out_sl", [TPC, D], F32, kind="ExternalOutput")
    dbg_t = {}
    if dbg:
        for nm, shp in [("d_sown", [1, TPC]), ("d_qT", [P, HPC, S]),
                        ("d_kT", [P, S]), ("d_cq", [1, S]), ("d_ck", [1, S]),
                        ("d_oT", [P, HPC, S]), ("d_x1T", [P, DMT, TPC]),
                        ("d_cmb", [E, TPC]), ("d_s2", [1, TPC])]:
            dbg_t[nm] = nc.dram_tensor(nm, shp, F32, kind="ExternalOutput")

    with tile.TileContext(nc) as tc, contextlib.ExitStack() as ctx:
        persist = ctx.enter_context(tc.tile_pool(name="persist", bufs=1))
        dram = ctx.enter_context(tc.tile_pool(name="dram", bufs=1, space="DRAM"))

        # ---------- persistent constants / long-lived small tiles ----------
        ones_sb = persist.tile([P, 1], F32R, tag="ones_sb")
        nc.sync.dma_start(ones_sb[:], ones_in.ap())
        ones16_sb = persist.tile([P, 1], F16, tag="ones16_sb")
        nc.sync.dma_start(ones16_sb[:], ones16_in.ap())
        idf_sb = persist.tile([P, P], F32, tag="idf_sb")
        nc.sync.dma_start(idf_sb[:], id_f.ap())
        qnw_sb = persist.tile([P, HPC], F32, tag="qnw_sb")
        nc.sync.dma_start(qnw_sb[:], qnw_in.ap())
        knw_sb = persist.tile([P, 1], F32, tag="knw_sb")
        nc.sync.dma_start(knw_sb[:], knw_in.ap())
        rb_sb = persist.tile([E, 1], F32, tag="rb_sb")
        nc.sync.dma_start(rb_sb[:], rbias.ap())
        xT_own = persist.tile([P, DKT, TPC], F32, tag="xT_own")
        nc.sync.dma_start(xT_own[:], x_slT.ap())
        x1T = persist.tile([P, DMT, TPC], F32, tag="x1T")
        h2b = persist.tile([P, DKT, TPC], BF16, tag="h2b")
        cmb_row = persist.tile([1, E, TPC], BF16, tag="cmb_row")

        def newton_rsqrt(row_ap, tmp_pool, n):
            """row <- refined 1/sqrt(u) given row holding u (in fp32)."""
            u = tmp_pool.tile([1, n], F32, tag="nr_u")
            nc.vector.tensor_copy(u[:], row_ap)
            nc.scalar.activation(row_ap, row_ap, AF.Sqrt)
            nc.vector.reciprocal(row_ap, row_ap)
            t = tmp_pool.tile([1, n], F32, tag="nr_t")
            nc.vector.tensor_mul(t[:], u[:], row_ap)
            nc.vector.tensor_mul(t[:], t[:], row_ap)
            nc.vector.tensor_scalar(t[:], t[:], -0.5, 1.5, OP.mult, OP.add)
            nc.vector.tensor_mul(row_ap, row_ap, t[:])

        def sumsq2(dst_ps, src_ap, pool, shape, first, last, from_psum=False):
            """dst_ps[1,n] (+)= column sums of src^2, hi/lo compensated."""
            p_, n_ = shape
            sqf = pool.tile([p_, n_], F32, tag="ss_f")
            nc.scalar.activation(sqf[:], src_ap, AF.Square)
            sqh = pool.tile([p_, n_], F32R, tag="ss_h")
            nc.vector.tensor_copy(sqh[:], sqf[:])
            sql = pool.tile([p_, n_], F32R, tag="ss_l")
            nc.vector.tensor_sub(sql[:], sqf[:], sqh[:])
            nc.tensor.matmul(dst_ps[:], ones_sb[:], sqh[:], start=first, stop=False)
            nc.tensor.matmul(dst_ps[:], ones_sb[:], sql[:], start=False, stop=last)

        with (
            tc.tile_pool(name="pAtt", bufs=1) as pAtt,   # q/k splits, v_tm: ph3-6
        ):
            mask_sb = pAtt.tile([P, 4, 512], F32, tag="mask_sb")
            nc.sync.dma_start(mask_sb[:], mask_in.ap())

            # x is replicated as a full input on every core: no AllGather.
            # The ln1-rms sumsq row for ALL tokens is accumulated from the
            # same rhs tiles the qkv loop streams through SBUF.
            s_row = pAtt.tile([1, S], F32, tag="s_row")

            # ---------- phase 3: fused QKV projection (3-pass compensated) ----------
            qT = pAtt.tile([P, HPC, S], F32, tag="qT")
            kT = pAtt.tile([P, S], F32, tag="kT")
            with tc.tile_pool(name="pV", bufs=1) as pV:
                vT = pV.tile([P, S], F32, tag="vT")
                arin_q = pV.tile([1, S], F32, tag="arin_q")
                arin_k = pV.tile([1, S], F32, tag="arin_k")
                with (
                    tc.tile_pool(name="qkv_w", bufs=3) as qkv_w,
                    tc.tile_pool(name="qkv_rhs", bufs=4) as rhs_pool,
                    tc.tile_pool(name="qkv_sq", bufs=3) as qsq_pool,
                    tc.tile_pool(name="qkv_ps", bufs=1, space="PSUM") as qkv_ps,
                    tc.tile_pool(name="qs_ps", bufs=1, space="PSUM") as qs_ps,
                ):
                    for ch in range(NCH):
                        pts = [qkv_ps.tile([P, 512], F32, tag=f"qkvp{m}",
                                           name=f"qkvp{m}") for m in range(4)]
                        sacc = qs_ps.tile([1, 512], F32, tag="sacc")
                        for kd in range(DKT):
                            wh_t = qkv_w.tile([P, 512], F16, tag="wh_t")
                            nc.sync.dma_start(wh_t[:], wqkv_h.ap()[:, kd])
                            wl_t = qkv_w.tile([P, 512], F16, tag="wl_t")
                            nc.sync.dma_start(wl_t[:], wqkv_l.ap()[:, kd])
                            rhs = rhs_pool.tile([P, 512], F32, tag="rhs")
                            nc.sync.dma_start(rhs[:],
                                              xT_full.ap()[:, kd,
                                                           ch * 512:(ch + 1) * 512])
                            rhs2 = rhs[:]
                            sumsq2(sacc, rhs2, qsq_pool, (P, 512),
                                   first=(kd == 0), last=(kd == DKT - 1))
                            rh = rhs_pool.tile([P, 512], F16, tag="rh")
                            nc.vector.tensor_copy(rh[:], rhs2)
                            rl = rhs_pool.tile([P, 512], F16, tag="rl")
                            nc.vector.tensor_sub(rl[:], rhs2, rh[:])
                            for m in range(4):
                                cs_w = slice(m * P, (m + 1) * P)
                                nc.tensor.matmul(pts[m][:], wh_t[:, cs_w], rh[:],
                                                 start=(kd == 0), stop=False)
                                nc.tensor.matmul(pts[m][:], wh_t[:, cs_w], rl[:],
                                                 start=False, stop=False)
                                nc.tensor.matmul(pts[m][:], wl_t[:, cs_w], rh[:],
                                                 start=False,
                                                 stop=(kd == DKT - 1))
                        cs = slice(ch * 512, (ch + 1) * 512)
                        qacc = qs_ps.tile([1, 512], F32, tag="qacc")
                        kacc = qs_ps.tile([1, 512], F32, tag="kacc")
                        for m in range(2):
                            nc.vector.tensor_scalar_mul(qT[:, m, cs], pts[m][:],
                                                        1.0 / 64.0)
                            sumsq2(qacc, qT[:, m, cs], qsq_pool, (P, 512),
                                   first=(m == 0), last=(m == 1))
                        nc.vector.tensor_scalar_mul(kT[:, cs], pts[2][:], 1.0 / 64.0)
                        sumsq2(kacc, kT[:, cs], qsq_pool, (P, 512),
                               first=True, last=True)
                        nc.vector.tensor_scalar_mul(vT[:, cs], pts[3][:], 1.0 / 64.0)
                        nc.vector.tensor_copy(arin_q[:, cs], qacc[:])
                        nc.vector.tensor_copy(arin_k[:, cs], kacc[:])
                        nc.vector.tensor_scalar(s_row[:, cs], sacc[:], 1.0 / D, EPS,
                                                OP.mult, OP.add)

                # ---------- phase 4: AllReduce qk sumsq; scale rows ----------
                ar_in = dram.tile([2, S], F32, tag="ar_in")
                ar_out = dram.tile([2, S], F32, addr_space="Shared", tag="ar_out")
                nc.sync.dma_start(ar_in[0:1, :], arin_q[:])
                nc.sync.dma_start(ar_in[1:2, :], arin_k[:])
                nc.gpsimd.collective_compute("AllReduce", OP.add, replica_groups=RG8,
                                             ins=[ar_in.opt()], outs=[ar_out.opt()])
                with tc.tile_pool(name="p4", bufs=1) as p4:
                    cq = p4.tile([1, S], F32, tag="cq")
                    ck = p4.tile([1, S], F32, tag="ck")
                    with (
                        tc.tile_pool(name="p4r", bufs=1) as p4r,
                        tc.tile_pool(name="crow", bufs=1) as crow,
                    ):
                        arq_r = p4r.tile([1, S], F32, tag="arq_r")
                        nc.sync.dma_start(arq_r[:], ar_out[0:1, :])
                        ark_r = p4r.tile([1, S], F32, tag="ark_r")
                        nc.sync.dma_start(ark_r[:], ar_out[1:2, :])
                        newton_rsqrt(s_row[:], p4r, S)
                        sflat = s_row[:]
                        for (dst, arrow, mean_div, post) in (
                                (cq, arq_r, float(H * DH), ISQ_DH),
                                (ck, ark_r, float(2 * KVH * DH), 1.0)):
                            t1 = crow.tile([1, S], F32, tag="t1")
                            nc.vector.tensor_mul(t1[:], sflat, sflat)
                            nc.vector.tensor_mul(t1[:], t1[:], arrow[:])
                            nc.vector.tensor_scalar(t1[:], t1[:], 1.0 / mean_div, EPS,
                                                    OP.mult, OP.add)
                            newton_rsqrt(t1[:], crow, S)
                            nc.vector.tensor_mul(t1[:], t1[:], sflat)
                            nc.vector.tensor_scalar_mul(dst[:], t1[:], post)
                        # v has no qk-norm to cancel the ln1 rms: apply the
                        # per-token scale row to vT directly
                        bsv = crow.tile([P, S], F32, tag="bsv")
                        nc.gpsimd.partition_broadcast(bsv[:], sflat)
                        nc.vector.tensor_mul(vT[:], vT[:], bsv[:])
                    if dbg:
                        nc.gpsimd.dma_start(dbg_t["d_cq"].ap(), cq[:])
                        nc.gpsimd.dma_start(dbg_t["d_ck"].ap(), ck[:])

                    # ---------- phase 5: qk norms + rope (fp32); splits ----------
                    with (
                        tc.tile_pool(name="bc", bufs=1) as bcp,
                        tc.tile_pool(name="rope", bufs=1) as rp,
                    ):
                        cos_sb = rp.tile([RD, S], F32, tag="cos_sb")
                        nc.sync.dma_start(cos_sb[:], cos_in.ap())
                        sin_sb = rp.tile([RD, S], F32, tag="sin_sb")
                        nc.sync.dma_start(sin_sb[:], sin_in.ap())
                        bq = bcp.tile([P, S], F32, tag="bq")
                        nc.gpsimd.partition_broadcast(bq[:], cq[:])
                        bk = bcp.tile([P, S], F32, tag="bk")
                        nc.gpsimd.partition_broadcast(bk[:], ck[:])
                        for m in range(HPC):
                            nc.vector.tensor_mul(qT[:, m, :], qT[:, m, :], bq[:])
                            nc.vector.tensor_scalar_mul(qT[:, m, :], qT[:, m, :],
                                                        qnw_sb[:, m:m + 1])
                        nc.vector.tensor_mul(kT[:], kT[:], bk[:])
                        nc.vector.tensor_scalar_mul(kT[:], kT[:], knw_sb[:, 0:1])
                        HF = RD // 2
                        for ap_ in [qT[:, 0, :], qT[:, 1, :], kT[:]]:
                            # qsh = rotate_half layout: [q[HF:RD]; q[0:HF]]
                            qsh = rp.tile([RD, S], F32, tag="qsh")
                            nc.sync.dma_start(qsh[0:HF, :], ap_[HF:RD, :])
                            nc.sync.dma_start(qsh[HF:RD, :], ap_[0:HF, :])
                            # sin table has rows 0:HF pre-negated on host
                            nc.vector.tensor_mul(qsh[0:RD, :], qsh[0:RD, :],
                                                 sin_sb[0:RD, :])
                            nc.vector.tensor_mul(ap_[0:RD, :], ap_[0:RD, :],
                                                 cos_sb[0:RD, :])
                            nc.vector.tensor_add(ap_[0:RD, :], ap_[0:RD, :],
                                                 qsh[0:RD, :])

                if dbg:
                    nc.gpsimd.dma_start(dbg_t["d_qT"].ap(), qT[:])
                    nc.gpsimd.dma_start(dbg_t["d_kT"].ap(), kT[:])

                # fp16 hi/lo splits of roped k
                kh16 = pAtt.tile([P, S], F16, tag="kh16")
                kl16 = pAtt.tile([P, S], F16, tag="kl16")
                nc.vector.tensor_copy(kh16[:], kT[:])
                nc.vector.tensor_sub(kl16[:], kT[:], kh16[:])

                # v to token-major (fp16 hi/lo)
                vh16 = pAtt.tile([P, DKT, DH], F16, tag="vh16")
                vl16 = pAtt.tile([P, DKT, DH], F16, tag="vl16")
                with (
                    tc.tile_pool(name="vt_ps", bufs=2, space="PSUM") as vt_ps,
                    tc.tile_pool(name="vt_sb", bufs=2) as vt_sb,
                ):
                    for kt in range(DKT):
                        pt = vt_ps.tile([P, P], F32, tag="vt")
                        nc.tensor.transpose(pt[:], vT[:, kt * P:(kt + 1) * P],
                                            idf_sb[:])
                        nc.vector.tensor_copy(vh16[:, kt, :], pt[:])
                        nc.vector.tensor_sub(vl16[:, kt, :], pt[:], vh16[:, kt, :])

            # ---------- phase 6: attention (compensated scores + av) ----------
            with tc.tile_pool(name="pO", bufs=1) as pO:
                oT = pO.tile([P, HPC, S], F32, tag="oT")
                with (
                    tc.tile_pool(name="sc_ps", bufs=2, space="PSUM") as sc_ps,
                    tc.tile_pool(name="o_ps", bufs=2, space="PSUM") as o_ps,
                    tc.tile_pool(name="sm_ps", bufs=2, space="PSUM") as sm_ps,
                    tc.tile_pool(name="eT", bufs=4) as e_pool,
                    tc.tile_pool(name="att_sb", bufs=2) as att_sb,
                ):
                    for m in range(HPC):
                        for qc in range(NCH):
                            nkt = 4 * qc + 4
                            qsl = slice(qc * 512, (qc + 1) * 512)
                            qh = att_sb.tile([P, 512], F16, tag="qh")
                            nc.vector.tensor_copy(qh[:], qT[:, m, qsl])
                            ql = att_sb.tile([P, 512], F16, tag="ql")
                            nc.vector.tensor_sub(ql[:], qT[:, m, qsl], qh[:])
                            opsum = o_ps.tile([P, 512], F32, tag="o")
                            spsum = sm_ps.tile([1, 512], F32, tag="s")
                            for kt in range(nkt):
                                ks = slice(kt * P, (kt + 1) * P)
                                scp = sc_ps.tile([P, 512], F32, tag="sc")
                                nc.tensor.matmul(scp[:], kh16[:, ks], qh[:],
                                                 start=True, stop=False)
                                nc.tensor.matmul(scp[:], kh16[:, ks], ql[:],
                                                 start=False, stop=False)
                                nc.tensor.matmul(scp[:], kl16[:, ks], qh[:],
                                                 start=False, stop=True)
                                eT = e_pool.tile([P, 512], F32, tag="e")
                                nc.scalar.activation(eT[:], scp[:], AF.Exp)
                                if kt >= 4 * qc:
                                    # mask table pre-scaled by 0.25 on host
                                    nc.vector.tensor_mul(eT[:], eT[:],
                                                         mask_sb[:, kt - 4 * qc, :])
                                else:
                                    nc.vector.tensor_scalar_mul(eT[:], eT[:], 0.25)
                                eh = e_pool.tile([P, 512], F16, tag="eh")
                                nc.vector.tensor_copy(eh[:], eT[:])
                                el = e_pool.tile([P, 512], F16, tag="el")
                                nc.vector.tensor_sub(el[:], eT[:], eh[:])
                                nc.tensor.matmul(spsum[:], ones16_sb[:], eh[:],
                                                 start=(kt == 0), stop=False)
                                nc.tensor.matmul(spsum[:], ones16_sb[:], el[:],
                                                 start=False, stop=(kt == nkt - 1))
                                nc.tensor.matmul(opsum[:], vh16[:, kt, :], eh[:],
                                                 start=(kt == 0), stop=False)
                                nc.tensor.matmul(opsum[:], vh16[:, kt, :], el[:],
                                                 start=False, stop=False)
                                nc.tensor.matmul(opsum[:], vl16[:, kt, :], eh[:],
                                                 start=False, stop=(kt == nkt - 1))
                            rrow = att_sb.tile([1, 512], F32, tag="rr")
                            nc.vector.reciprocal(rrow[:], spsum[:])
                            brr = att_sb.tile([P, 512], F32, tag="brr")
                            nc.gpsimd.partition_broadcast(brr[:], rrow[:])
                            nc.vector.tensor_mul(oT[:, m, qsl], opsum[:], brr[:])

                if dbg:
                    nc.gpsimd.dma_start(dbg_t["d_oT"].ap(), oT[:])
                # ---------- phase 7: AllToAll o^T -> token-sharded ----------
                a2a_in = dram.tile([NCORE, HPC * P, TPC], F32, tag="a2a_in")
                a2a_out = dram.tile([NCORE, HPC * P, TPC], F32, tag="a2a_out")
                for j in range(NCORE):
                    nc.sync.dma_start(a2a_in[j].rearrange("(m p) u -> p m u", p=P),
                                      oT[:, :, j * TPC:(j + 1) * TPC])
                nc.gpsimd.collective_compute("AllToAll", OP.bypass, replica_groups=RG8,
                                             ins=[a2a_in.opt()], outs=[a2a_out.opt()])
            oTo = a2a_out.rearrange("r q u -> (r q) u")   # [H*DH, TPC] global odim rows

            # ---------- phase 8: o-proj (3-pass) + residual -> x1 (fp32) ----------
            with (
                tc.tile_pool(name="p8", bufs=1) as p8,
                tc.tile_pool(name="wo_str", bufs=4) as wo_str,
                tc.tile_pool(name="op_ps", bufs=2, space="PSUM") as op_ps,
            ):
                oTo_f = p8.tile([P, DKT, TPC], F32, tag="oTo_f")
                nc.sync.dma_start(oTo_f[:], oTo.rearrange("(ko p) u -> p ko u", p=P))
                oh16 = p8.tile([P, DKT, TPC], F16, tag="oh16")
                ol16 = p8.tile([P, DKT, TPC], F16, tag="ol16")
                for ko in range(DKT):
                    nc.vector.tensor_copy(oh16[:, ko, :], oTo_f[:, ko, :])
                    nc.vector.tensor_sub(ol16[:, ko, :], oTo_f[:, ko, :],
                                         oh16[:, ko, :])
                for md in range(DMT):
                    wth = wo_str.tile([P, DKT, P], F16, tag="wth")
                    nc.sync.dma_start(wth[:], wo_h.ap()[:, md])
                    wtl = wo_str.tile([P, DKT, P], F16, tag="wtl")
                    nc.sync.dma_start(wtl[:], wo_l.ap()[:, md])
                    pt = op_ps.tile([P, TPC], F32, tag="op")
                    for ko in range(DKT):
                        nc.tensor.matmul(pt[:], wth[:, ko, :], oh16[:, ko, :],
                                         start=(ko == 0), stop=False)
                        nc.tensor.matmul(pt[:], wth[:, ko, :], ol16[:, ko, :],
                                         start=False, stop=False)
                        nc.tensor.matmul(pt[:], wtl[:, ko, :], oh16[:, ko, :],
                                         start=False, stop=(ko == DKT - 1))
                    nc.vector.scalar_tensor_tensor(x1T[:, md, :], pt[:], 1.0 / 64.0,
                                                   xT_own[:, md, :], OP.mult, OP.add)

        if dbg:
            nc.gpsimd.dma_start(dbg_t["d_x1T"].ap(), x1T[:])
        # ---------- phase 9: ln2 rms, h2, router, combine ----------
        with (
            tc.tile_pool(name="p9", bufs=1) as p9,
            tc.tile_pool(name="s2_sb", bufs=3) as s2_pool,
            tc.tile_pool(name="s2_ps", bufs=1, space="PSUM") as s2_ps,
            tc.tile_pool(name="rt_sb", bufs=1) as rt_sb,
            tc.tile_pool(name="rt_ps", bufs=1, space="PSUM") as rt_ps,
        ):
            s2row = p9.tile([1, TPC], F32, tag="s2row")
            acc2 = s2_ps.tile([1, TPC], F32, tag="acc2")
            for kd in range(DKT):
                sumsq2(acc2, x1T[:, kd, :], s2_pool, (P, TPC),
                       first=(kd == 0), last=(kd == DKT - 1))
            nc.vector.tensor_scalar(s2row[:], acc2[:], 1.0 / D, EPS, OP.mult, OP.add)
            newton_rsqrt(s2row[:], s2_pool, TPC)

            h2f = p9.tile([P, DKT, TPC], F32, tag="h2f")
            bs2 = p9.tile([P, TPC], F32, tag="bs2")
            nc.gpsimd.partition_broadcast(bs2[:], s2row[:])
            for kd in range(DKT):
                nc.vector.tensor_mul(h2f[:, kd, :], x1T[:, kd, :], bs2[:])
                nc.vector.tensor_copy(h2b[:, kd, :], h2f[:, kd, :])
            h2h16 = p9.tile([P, DKT, TPC], F16, tag="h2h16")
            h2l16 = p9.tile([P, DKT, TPC], F16, tag="h2l16")
            for kd in range(DKT):
                nc.vector.tensor_copy(h2h16[:, kd, :], h2f[:, kd, :])
                nc.vector.tensor_sub(h2l16[:, kd, :], h2f[:, kd, :],
                                     h2h16[:, kd, :])
            rwh_sb = rt_sb.tile([P, DKT, E], F16, tag="rwh_sb")
            nc.sync.dma_start(rwh_sb[:], rw_h.ap())
            rwl_sb = rt_sb.tile([P, DKT, E], F16, tag="rwl_sb")
            nc.sync.dma_start(rwl_sb[:], rw_l.ap())
            lg = rt_ps.tile([E, TPC], F32, tag="lg")
            for kd in range(DKT):
                nc.tensor.matmul(lg[:], rwh_sb[:, kd, :], h2h16[:, kd, :],
                                 start=(kd == 0), stop=False)
                nc.tensor.matmul(lg[:], rwh_sb[:, kd, :], h2l16[:, kd, :],
                                 start=False, stop=False)
                nc.tensor.matmul(lg[:], rwl_sb[:, kd, :], h2h16[:, kd, :],
                                 start=False, stop=(kd == DKT - 1))
            lgf = rt_sb.tile([E, TPC], F32, tag="lgf")
            nc.vector.tensor_scalar_mul(lgf[:], lg[:], 1.0 / 64.0)
            sg = rt_sb.tile([E, TPC], F32, tag="sg")
            nc.scalar.activation(sg[:], lgf[:], AF.Sigmoid)
            biased = rt_sb.tile([E, TPC], F32, tag="biased")
            nc.vector.tensor_scalar_add(biased[:], sg[:], rb_sb[:, 0:1])
            m1 = rt_sb.tile([E, TPC], F32, tag="m1")
            nc.gpsimd.partition_all_reduce(m1[:], biased[:], channels=E,
                                           reduce_op=bass_isa.ReduceOp.max)
            eq = rt_sb.tile([E, TPC], F32, tag="eq")
            nc.vector.tensor_tensor(eq[:], biased[:], m1[:], OP.is_equal)
            nc.vector.tensor_scalar_mul(eq[:], eq[:], -1e9)
            nc.vector.tensor_add(eq[:], eq[:], biased[:])
            m2 = rt_sb.tile([E, TPC], F32, tag="m2")
            nc.gpsimd.partition_all_reduce(m2[:], eq[:], channels=E,
                                           reduce_op=bass_isa.ReduceOp.max)
            ind = rt_sb.tile([E, TPC], F32, tag="ind")
            nc.vector.tensor_tensor(ind[:], biased[:], m2[:], OP.is_ge)
            aff = rt_sb.tile([E, TPC], F32, tag="aff")
            nc.vector.tensor_mul(aff[:], sg[:], ind[:])
            den = rt_sb.tile([E, TPC], F32, tag="den")
            nc.gpsimd.partition_all_reduce(den[:], aff[:], channels=E,
                                           reduce_op=bass_isa.ReduceOp.add)
            rden = rt_sb.tile([E, TPC], F32, tag="rden")
            nc.vector.reciprocal(rden[:], den[:])
            nc.vector.tensor_mul(aff[:], aff[:], rden[:])
            cmb_bf = rt_sb.tile([E, TPC], BF16, tag="cmb_bf")
            nc.vector.tensor_copy(cmb_bf[:], aff[:])
            nc.sync.dma_start(cmb_row[:], cmb_bf[:])
            if dbg:
                nc.gpsimd.dma_start(dbg_t["d_cmb"].ap(), aff[:])
                nc.gpsimd.dma_start(dbg_t["d_s2"].ap(), s2row[:])

        # ---------- phase 10: dense MoE over all experts (bf16) ----------
        with (
            tc.tile_pool(name="p10", bufs=1) as p10,
            tc.tile_pool(name="wmoe", bufs=4) as wmoe,
            tc.tile_pool(name="moe_ps", bufs=2, space="PSUM") as moe_ps,
            tc.tile_pool(name="moe_sb", bufs=2) as moe_sb,
        ):
            act_all = p10.tile([P, E, IMT, TPC], BF16, tag="act_all")
            up_bf = p10.tile([P, IMT, TPC], BF16, tag="up_bf")
            out_fm = p10.tile([P, DMT, TPC], F32, tag="out_fm")
            for e in range(E):
                bce = moe_sb.tile([P, TPC], BF16, tag="bce")
                nc.gpsimd.partition_broadcast(bce[:], cmb_row[0:1, e, :])
                for mi in range(IMT):
                    wt = wmoe.tile([P, DKT, P], BF16, tag="wmu")
                    nc.gpsimd.dma_start(wt[:], wu_p.ap()[:, e, mi])
                    pt = moe_ps.tile([P, TPC], F32, tag="up")
                    for kd in range(DKT):
                        nc.tensor.matmul(pt[:], wt[:, kd, :], h2b[:, kd, :],
                                         start=(kd == 0), stop=(kd == DKT - 1))
                    nc.vector.tensor_copy(up_bf[:, mi, :], pt[:])
                for mi in range(IMT):
                    wt = wmoe.tile([P, DKT, P], BF16, tag="wmg")
                    nc.gpsimd.dma_start(wt[:], wg_p.ap()[:, e, mi])
                    pt = moe_ps.tile([P, TPC], F32, tag="gate")
                    for kd in range(DKT):
                        nc.tensor.matmul(pt[:], wt[:, kd, :], h2b[:, kd, :],
                                         start=(kd == 0), stop=(kd == DKT - 1))
                    gs = moe_sb.tile([P, TPC], BF16, tag="gs")
                    nc.scalar.activation(gs[:], pt[:], AF.Silu)
                    nc.vector.tensor_mul(gs[:], gs[:], up_bf[:, mi, :])
                    nc.vector.tensor_mul(act_all[:, e, mi, :], gs[:], bce[:])
            for md in range(DMT):
                pt = moe_ps.tile([P, TPC], F32, tag="dn")
                for e in range(E):
                    wt = wmoe.tile([P, IKT, P], BF16, tag="wmd")
                    nc.gpsimd.dma_start(wt[:], wd_p.ap()[:, e, md])
                    for ki in range(IKT):
                        nc.tensor.matmul(pt[:], wt[:, ki, :], act_all[:, e, ki, :],
                                         start=(e == 0 and ki == 0),
                                         stop=(e == E - 1 and ki == IKT - 1))
                nc.vector.tensor_add(out_fm[:, md, :], pt[:], x1T[:, md, :])

            # ---------- phase 11: transpose to token-major; write output ----------
            out_tm = p10.tile([P, 2, DMT, P], F32, tag="out_tm")
            with tc.tile_pool(name="ot_ps", bufs=2, space="PSUM") as ot_ps:
                for md in range(DMT):
                    for tb in range(2):
                        pt = ot_ps.tile([P, P], F32, tag="ot")
                        nc.tensor.transpose(pt[:], out_fm[:, md, tb * P:(tb + 1) * P],
                                            idf_sb[:])
                        nc.vector.tensor_copy(out_tm[:, tb, md, :], pt[:])
            nc.sync.dma_start(
                out_sl.ap().rearrange("(tb p) (md c) -> p tb md c", p=P, c=P),
                out_tm[:])

    nc.compile()
    return nc


# ======================================================================
# host-side input preparation
# ======================================================================

def _trunc_hi(w, bits=12):
    """Zero all but the top `bits` mantissa bits (hi half survives fp32r rounding)."""
    u = np.ascontiguousarray(w, dtype=np.float32).view(np.uint32)
    mask = np.uint32(0xFFFFFFFF) << np.uint32(23 - bits)
    return (u & mask).view(np.float32)


def prep_in_maps(inputs):
    f32 = lambda a: np.ascontiguousarray(np.asarray(a), dtype=np.float32)
    x = f32(inputs["x"]).reshape(S, D)
    ln1 = f32(inputs["ln1_w"])
    ln2 = f32(inputs["ln2_w"])
    wq = f32(inputs["wq"]) * ln1[:, None]
    wk = f32(inputs["wk"]) * ln1[:, None]
    wv = f32(inputs["wv"]) * ln1[:, None]
    wo = f32(inputs["wo"])
    qnw = f32(inputs["qnorm_w"])
    knw = f32(inputs["knorm_w"])
    rw = f32(inputs["router_w"]) * ln2[:, None]
    rb = f32(inputs["router_bias"]).reshape(E, 1)
    wg = f32(inputs["wg"]) * ln2[None, :, None]
    wu = f32(inputs["wu"]) * ln2[None, :, None]
    wd = f32(inputs["wd"])

    # The rope tables must BIT-MATCH the fp32 jax reference: at angles of
    # ~2000 rad a 1-ulp difference in inv_freq moves cos/sin by ~1e-4, which
    # perturbs x1 enough to flip near-tie router decisions.  So compute them
    # with jax itself on the CPU backend, mirroring the reference ops.
    import jax as _jax
    import jax.numpy as _jnp
    with _jax.default_device(_jax.devices("cpu")[0]):
        _pos = _jnp.arange(S, dtype=_jnp.float32)
        _invf = 1.0 / (1000000.0 ** (_jnp.arange(0, RD, 2, dtype=_jnp.float32) / RD))
        _ang = _pos[:, None] * _invf[None, :]
        _emb = _jnp.concatenate([_ang, _ang], axis=-1)       # [S, RD]
        cos_t = np.ascontiguousarray(np.asarray(_jnp.cos(_emb)).T)  # [RD, S]
        sin_t = np.ascontiguousarray(np.asarray(_jnp.sin(_emb)).T)
    sin_t[:RD // 2] *= -1.0   # fold rotate_half sign into the table

    ident = np.eye(P, dtype=np.float32)
    ones_c = np.ones((P, 1), dtype=np.float32)
    p_i = np.arange(P)[:, None, None]
    off_i = np.arange(4)[None, :, None]
    q_i = np.arange(512)[None, None, :]
    mask = ((P * off_i + p_i) <= q_i).astype(np.float32) * 0.25

    pack_kd = lambda w: np.ascontiguousarray(
        w.reshape(DKT, P, w.shape[1]).transpose(1, 0, 2))   # [D, C] -> [P, DKT, C]

    def f16_split(w):
        ws = (w * 64.0).astype(np.float32)
        hi = ws.astype(np.float16)
        lo = (ws - hi.astype(np.float32)).astype(np.float16)
        return hi, lo

    bf = ml_dtypes.bfloat16
    wg_pk = np.ascontiguousarray(
        wg.reshape(E, DKT, P, IMT, P).transpose(2, 0, 3, 1, 4).astype(bf))
    wu_pk = np.ascontiguousarray(
        wu.reshape(E, DKT, P, IMT, P).transpose(2, 0, 3, 1, 4).astype(bf))
    wd_pk = np.ascontiguousarray(
        wd.reshape(E, IKT, P, DMT, P).transpose(2, 0, 3, 1, 4).astype(bf))
    wo_hi, wo_lo = f16_split(wo)
    pack_wo = lambda w: np.ascontiguousarray(
        w.reshape(DKT, P, DMT, P).transpose(1, 2, 0, 3))
    wo_hp = pack_wo(wo_hi)
    wo_lp = pack_wo(wo_lo)
    rw_hi, rw_lo = f16_split(rw)

    in_maps = []
    for c in range(NCORE):
        qcols = slice(c * HPC * DH, (c + 1) * HPC * DH)
        kvcols = slice((c // 2) * DH, (c // 2 + 1) * DH)
        wqkv_c = np.concatenate([wq[:, qcols], wk[:, kvcols], wv[:, kvcols]], axis=1)
        wqkv_hi, wqkv_lo = f16_split(wqkv_c)
        qnw_c = np.ascontiguousarray(qnw[qcols].reshape(HPC, P).T)
        knw_c = np.ascontiguousarray(knw[kvcols].reshape(1, P).T)
        x_sl = x[c * TPC:(c + 1) * TPC]                      # [TPC, D]
        x_slT_c = np.ascontiguousarray(
            x_sl.T.reshape(DKT, P, TPC).transpose(1, 0, 2))  # [P, DKT, TPC]
        if c == 0:
            xT_full_c = np.ascontiguousarray(
                x.T.reshape(DKT, P, S).transpose(1, 0, 2))   # [P, DKT, S]
        in_maps.append({
            "x_slT": x_slT_c,
            "xT_full": xT_full_c,
            "wqkv_h": pack_kd(wqkv_hi),
            "wqkv_l": pack_kd(wqkv_lo),
            "wo_h": wo_hp,
            "wo_l": wo_lp,
            "rw_h": pack_kd(rw_hi),
            "rw_l": pack_kd(rw_lo),
            "ones16_in": ones_c.astype(np.float16),
            "rbias": rb,
            "cos_in": cos_t,
            "sin_in": sin_t,
            "id_f": ident,
            "ones_in": ones_c,
            "qnw_in": qnw_c,
            "knw_in": knw_c,
            "mask_in": mask,
            "wg_p": wg_pk,
            "wu_p": wu_pk,
            "wd_p": wd_pk,
        })
    return in_maps


_CACHE = {}


def get_module():
    if "nc" not in _CACHE:
        _CACHE["nc"] = build_module()
    return _CACHE["nc"]


def kernel(**inputs) -> np.ndarray:
    nc = get_module()
    in_maps = prep_in_maps(inputs)
    res = bass_utils.run_bass_kernel_spmd(nc, in_maps, core_ids=list(range(NCORE)))
    out = np.concatenate([res.results[c]["out_sl"] for c in range(NCORE)], axis=0)
    return out.reshape(1, S, D).astype(np.float32)


if __name__ == "__main__":
    build_module()
    print("module built ok")
